# revision 30
# baseline (speedup 1.0000x reference)
"""Multi-head self-attention (B=4, S=2048, D=1024, H=16) on 8 NeuronCores.

Sharding: data-parallel over batch (4 groups) x tensor-parallel over heads
(2 groups of 8 heads).  Core c handles batch b=c//2, head-group g=c%2.
Each core computes its 8 heads' attention plus a partial out-projection;
the host sums the two partials per batch, transposes, adds out_b.

v2 design (vs v1): everything in bf16 (plenty of margin vs the 2e-2 gate),
no DRAM staging, and a "flipped" AV matmul that halves the PE rows:

  - scores^T per head-pair via row-packed K=64 matmuls, psum tile
    [128 keys, 2*CH] holds both heads; ONE exp per ktile ([128, 1024]
    activation, scale=1/8 folded) -> pt [keys, q] bf16 in SBUF
  - AV flipped: stationary = pt q-block [128 keys, 128 q], moving = v_aug
    [128 keys, 65] (64 v dims + ones column) -> psum [128 q, 65] per
    (qblock, head), accumulated over the 16 ktiles.  65 moving rows per
    ktile instead of 128 q rows: ~2x fewer PE cycles than v1's AV.  The
    ones column (installed by a broadcast bias add on the v projection)
    lands the softmax denominator in psum column 64, per PARTITION
    (= per q), so normalization is a native tensor_scalar multiply.
  - PSUM accumulation groups must run ONE AT A TIME per psum bank
    (a start=True while another group is open in the same bank resets the
    bank - verified on hw).  So AV for chunk (c) runs as a post-pass
    (one (qblock, head) group after another) in the NEXT chunk's window,
    with pt double-buffered.
  - o comes out [q, feat]; out-projection needs o^T, done with cheap PE
    transposes (128 rows each) through PSUM.
  - schedule: pair-outer; pair p+1's q/k/v projections and chunk c-1's
    out-projection interleave into pair p's ACT-bound attention windows.
    ACT does only the 256 exps (~266us); PE ~280us; DVE does all
    PSUM->SBUF moves + bias/normalize (~110us).  gpsimd cannot read PSUM
    (walrus codegen fails) so DVE carries the copies.
"""

import numpy as np

_B, _S, _D, _H = 4, 2048, 1024, 16
_FH = 512        # local feature dims per core (8 heads x 64)
_ND = _D // 128  # contraction tiles
_NPAIR = 4       # head pairs (2 heads of 64 -> 128 features)
_NKT = _S // 128 # key tiles
_CH = 512        # q chunk
_NCH = _S // _CH
_NQB = _CH // 128
_NH = 8          # local heads
_FHA = _NH * 65  # v width incl. per-head ones column
_NCORES = 8

_CACHE = {}
_DEBUG = False  # adds qkT/v/o debug outputs to the kernel


def _build():
    import concourse.bass as bass
    import concourse.bacc as bacc
    import concourse.tile as tile
    import concourse.mybir as mybir
    from contextlib import ExitStack

    f32 = mybir.dt.float32
    bf16 = mybir.dt.bfloat16
    Exp = mybir.ActivationFunctionType.Exp
    D, S, FH, ND, NPAIR, NKT, CH, NCH, NQB, FHA = (
        _D, _S, _FH, _ND, _NPAIR, _NKT, _CH, _NCH, _NQB, _FHA)

    nc = bacc.Bacc("TRN2", target_bir_lowering=False, debug=False)

    xP_d = nc.dram_tensor("xP", [NCH, 128, ND, CH], bf16, kind="ExternalInput")
    wq_d = nc.dram_tensor("wq", [NPAIR, 128, ND, 128], bf16, kind="ExternalInput")
    wk_d = nc.dram_tensor("wk", [NPAIR, 128, ND, 128], bf16, kind="ExternalInput")
    wv_d = nc.dram_tensor("wv", [128, ND, FHA], bf16, kind="ExternalInput")
    wo_d = nc.dram_tensor("wo", [128, NPAIR, D], bf16, kind="ExternalInput")
    bq_d = nc.dram_tensor("bq", [128, NPAIR], f32, kind="ExternalInput")
    bk_d = nc.dram_tensor("bk", [128, NPAIR], f32, kind="ExternalInput")
    bv_d = nc.dram_tensor("bv", [1, FHA], bf16, kind="ExternalInput")
    idn_d = nc.dram_tensor("idn", [128, 128], bf16, kind="ExternalInput")
    outp_d = nc.dram_tensor("outp", [NCH, ND, 128, CH], bf16, kind="ExternalOutput")
    if _DEBUG:
        dbg_qkT = nc.dram_tensor("dbg_qkT", [128, NPAIR, 2, S], bf16, kind="ExternalOutput")
        dbg_v = nc.dram_tensor("dbg_v", [128, NKT, FHA], bf16, kind="ExternalOutput")
        dbg_o = nc.dram_tensor("dbg_o", [128, S // 128, FH], bf16, kind="ExternalOutput")

    with tile.TileContext(nc) as tc, ExitStack() as top:
        consts = top.enter_context(tc.tile_pool(name="consts", bufs=1))
        ps = top.enter_context(tc.tile_pool(name="ps", bufs=2, space="PSUM"))
        big = top.enter_context(tc.tile_pool(name="big", bufs=1))
        ptp = top.enter_context(tc.tile_pool(name="ptp", bufs=2))
        wst = top.enter_context(tc.tile_pool(name="wst", bufs=2))
        otp = top.enter_context(tc.tile_pool(name="otp", bufs=2))
        stp = top.enter_context(tc.tile_pool(name="stp", bufs=3))
        rcpp = top.enter_context(tc.tile_pool(name="rcpp", bufs=2))

        xT_sb = big.tile([128, ND, S], bf16)
        qkT = big.tile([128, NPAIR, 2, S], bf16)  # [feat%128, pair, q/k, t]
        v_sb = big.tile([128, NKT, FHA], bf16)    # [token%128, ktile, head*65]
        o_sb = big.tile([128, S // 128, FH], bf16)  # [q%128, qblock, feat]
        wv_sb = big.tile([128, ND, FHA], bf16)
        wo_sb = big.tile([128, NPAIR, D], bf16)

        def load_w(p):
            wq_sb = wst.tile([128, ND, 128], bf16, tag="wq")
            nc.sync.dma_start(out=wq_sb, in_=wq_d[p])
            wk_sb = wst.tile([128, ND, 128], bf16, tag="wk")
            nc.sync.dma_start(out=wk_sb, in_=wk_d[p])
            return wq_sb, wk_sb

        # DMA order tuned for the warmup critical path: the first qkproj
        # half-slice needs the first half of x slice 0 + wq0/wk0; the
        # first vproj needs wv
        w_cur = load_w(0)
        nc.sync.dma_start(out=xT_sb[:, :, 0:CH // 2], in_=xP_d[0][:, :, 0:CH // 2])
        nc.sync.dma_start(out=xT_sb[:, :, CH // 2:CH], in_=xP_d[0][:, :, CH // 2:CH])
        nc.sync.dma_start(out=wv_sb, in_=wv_d[:])
        bqk_sb = consts.tile([128, 2 * NPAIR], f32)
        nc.sync.dma_start(out=bqk_sb[:, 0:NPAIR], in_=bq_d[:])
        nc.sync.dma_start(out=bqk_sb[:, NPAIR:2 * NPAIR], in_=bk_d[:])
        # v bias broadcast to all partitions (includes the 1.0 ones-column
        # entries that seed the softmax-denominator trick)
        bvb_sb = consts.tile([128, FHA], bf16)
        nc.sync.dma_start(out=bvb_sb, in_=bv_d[:].to_broadcast([128, FHA]))
        for ts in range(1, NCH):
            nc.sync.dma_start(
                out=xT_sb[:, :, ts * CH:(ts + 1) * CH], in_=xP_d[ts])
        idn_sb = consts.tile([128, 128], bf16)
        nc.sync.dma_start(out=idn_sb, in_=idn_d[:])
        nc.sync.dma_start(out=wo_sb, in_=wo_d[:])
        # dummy exp so the ACT table set loads during the ramp
        warm = consts.tile([1, 8], f32)
        nc.vector.memset(warm, 0.0)
        nc.scalar.activation(out=warm, in_=warm, func=Exp)

        def qkproj_slice(p, j, which, w_sb, halves=1):
            pps = ps.tile([128, CH], f32, tag="mix")
            hw_ = CH // halves
            for hf in range(halves):
                for d in range(ND):
                    nc.tensor.matmul(
                        pps[:, hf * hw_:(hf + 1) * hw_],
                        lhsT=w_sb[:, d, :],
                        rhs=xT_sb[:, d, j * CH + hf * hw_:
                                  j * CH + (hf + 1) * hw_],
                        start=(d == 0),
                        stop=(d == ND - 1),
                    )
            nc.vector.tensor_scalar_add(
                out=qkT[:, p, which, j * CH:(j + 1) * CH],
                in0=pps,
                scalar1=bqk_sb[:, which * NPAIR + p:which * NPAIR + p + 1],
            )

        def vproj_t(p, t):
            vps = ps.tile([128, 130], f32, tag="mix")
            for d in range(ND):
                nc.tensor.matmul(
                    vps,
                    lhsT=xT_sb[:, d, t * 128:(t + 1) * 128],
                    rhs=wv_sb[:, d, p * 130:(p + 1) * 130],
                    start=(d == 0),
                    stop=(d == ND - 1),
                )
            nc.vector.tensor_add(
                out=v_sb[:, t, p * 130:(p + 1) * 130],
                in0=vps,
                in1=bvb_sb[:, p * 130:(p + 1) * 130],
            )

        def score_unit(p, c, i, pt_cur):
            sAB = ps.tile([128, 2 * CH], f32, tag="sab")
            nc.tensor.matmul(
                sAB[:, 0:CH],
                lhsT=qkT[0:64, p, 1, i * 128:(i + 1) * 128],
                rhs=qkT[0:64, p, 0, c * CH:(c + 1) * CH],
                start=True, stop=True,
                tile_position=(0, 0),
            )
            nc.tensor.matmul(
                sAB[:, CH:2 * CH],
                lhsT=qkT[64:128, p, 1, i * 128:(i + 1) * 128],
                rhs=qkT[64:128, p, 0, c * CH:(c + 1) * CH],
                start=True, stop=True,
                tile_position=(64, 0),
            )
            nc.scalar.activation(
                out=pt_cur[:, i, :], in_=sAB, func=Exp, scale=0.125)

        def av_ktile(p, av_t, half, i, pt_cur, first, last):
            """AV matmuls for one psum bank (av01 or av23) at ktile i.

            The bank runs ONE accumulation context per chunk: start=True
            only on the bank's first write (resets the bank's
            written-bitmap; untouched regions then store on first touch,
            accumulate after -- verified on hw), stop on its last.
            """
            for qbl in range(2):
                qb = half * 2 + qbl
                for h in range(2):
                    nc.tensor.matmul(
                        av_t[half][:, qbl, h * 65:(h + 1) * 65],
                        lhsT=pt_cur[:, i, h * CH + qb * 128:
                                    h * CH + (qb + 1) * 128],
                        rhs=v_sb[:, i, p * 130 + h * 65:
                                 p * 130 + (h + 1) * 65],
                        start=(first and qbl == 0 and h == 0),
                        stop=(last and qbl == 1 and h == 1),
                        skip_group_check=True,
                    )

        def norm_chunk(p, c, av_t, rcp_t):
            for half in range(2):
                for h in range(2):
                    nc.vector.reciprocal_approx_fast(
                        out=rcp_t[:, half, 2 * h:2 * h + 1],
                        in_=av_t[half][:, 0:1, 64 + 65 * h:65 + 65 * h],
                    )
                    nc.vector.reciprocal_approx_fast(
                        out=rcp_t[:, half, 2 * h + 1:2 * h + 2],
                        in_=av_t[half][:, 1:2, 64 + 65 * h:65 + 65 * h],
                    )
                for qbl in range(2):
                    qb = half * 2 + qbl
                    for h in range(2):
                        nc.vector.tensor_scalar_mul(
                            out=o_sb[:, c * NQB + qb,
                                     p * 128 + h * 64:p * 128 + (h + 1) * 64],
                            in0=av_t[half][:, qbl, h * 65:h * 65 + 64],
                            scalar1=rcp_t[:, half,
                                          2 * h + qbl:2 * h + qbl + 1],
                        )

        def emit_out_units(c):
            """Transposes + out-projection for chunk c, as thunks.

            tps units for fb<3 depend only on pairs 0-2 (whose chunk-c
            norms ran long ago); fb==3 waits on pair 3's norm.
            """
            oT = otp.tile([128, NPAIR, CH], bf16, tag="ot")

            def tps_unit(fb):
                def go():
                    tps = ps.tile([128, NQB, 128], bf16, tag="mix")
                    for qb in range(NQB):
                        nc.tensor.transpose(
                            out=tps[:, qb, :],
                            in_=o_sb[:, c * NQB + qb, fb * 128:(fb + 1) * 128],
                            identity=idn_sb,
                        )
                    nc.vector.tensor_copy(out=oT[:, fb, :], in_=tps)
                return go

            def ops_unit(et, on_act):
                def go():
                    ops = ps.tile([128, CH], f32, tag="mix")
                    for pb in range(NPAIR):
                        nc.tensor.matmul(
                            ops,
                            lhsT=wo_sb[:, pb, et * 128:(et + 1) * 128],
                            rhs=oT[:, pb, :],
                            start=(pb == 0),
                            stop=(pb == NPAIR - 1),
                        )
                    st = stp.tile([128, CH], bf16, tag="st")
                    if on_act:
                        nc.scalar.copy(out=st, in_=ops)
                    else:
                        nc.vector.tensor_copy(out=st, in_=ops)
                    nc.sync.dma_start(out=outp_d[c, et], in_=st)
                return go

            units = [tps_unit(fb) for fb in range(NPAIR)]
            # toward the tail ACT gains slack: alternate the psum->sbuf
            # copies between ACT and DVE so the DVE queue (which also
            # carries the final norm) doesn't serialize the drain
            units += [ops_unit(et, c >= NCH - 2 and et % 2 == 0)
                      for et in range(ND)]
            return units

        # ----- main: pair-outer, chunk-inner.  Per ktile: scores+exp for
        # ktile i, AV for ktile i-1 (bank av01) and i-2 (bank av23) -- the
        # lag keeps the in-order PE from blocking on the just-issued exp.
        # Projection / out-projection filler work is cost-paced between
        # ktiles so the PE never starves while pacing behind ACT. -----
        pend_out = None    # chunk index awaiting emit_out (pair 3)
        carry = []         # (thunk, deadline-ktile) for window (3, 0)
        w_nxt = None
        for p in range(NPAIR):
            if p + 1 < NPAIR:
                w_nxt = load_w(p + 1)
                items = [(500, (lambda pp=p + 1, t=t: vproj_t(pp, t)))
                         for t in range(NKT)]
                items += [(1740, (lambda pp=p + 1, j=j, w=which,
                                  ws=w_nxt[which]:
                           qkproj_slice(pp, j, w, ws)))
                          for j in range(NCH) for which in range(2)]
                if p == 2:
                    # window (3,0) has no filler work of its own: carry
                    # pair 3's last qkproj slices there, each emitted
                    # before the ktile that first consumes its k-slice
                    carry = [(items[20][1], 4), (items[21][1], 6),
                             (items[22][1], 8), (items[23][1], 10)]
                    items = items[:20]
            else:
                items = []
            n_items = len(items)
            emitted = 0
            nwin = NCH if p > 0 else NCH - 1
            denom = max(1, nwin * NKT - 6)
            it_count = 0

            for c in range(NCH):
                # fillers: (cost_ns, thunk) of PE work to spread between
                # the score units
                fillers = []
                if pend_out is not None:
                    fillers += [(220, u) if k < NPAIR else (870, u)
                                for k, u in enumerate(emit_out_units(pend_out))]
                    pend_out = None
                last_units = None
                if p == NPAIR - 1 and c == NCH - 1:
                    # the final chunk's fb 0..2 transposes only need pairs
                    # 0-2 (normalized long ago): run them in this window,
                    # leaving just tps3 + out-proj for the tail
                    last_units = emit_out_units(NCH - 1)
                    fillers += [(220, u) for u in last_units[:NPAIR - 1]]
                if items and not (p == 0 and c == 0):
                    it_count += NKT
                    want = min(n_items, (it_count * n_items) // denom)
                    while emitted < want:
                        fillers.append(items[emitted])
                        emitted += 1

                pt_cur = ptp.tile([128, NKT, 2 * CH], bf16, tag="pt")
                av_t = [ps.tile([128, 2, 130], f32, tag="av", name=f"av{h}")
                        for h in range(2)]
                rcp_t = rcpp.tile([128, 2, 4], f32, tag="rcp")
                total = sum(cn for cn, _ in fillers)
                spent = 0
                for i in range(NKT):
                    if p == 0 and c == 0:
                        # inline projections for pair 0, aligned with the
                        # ktile order scores consume them in; the first
                        # slices run half-width so the leading matmuls only
                        # wait on the first half-slice x DMA
                        if i % 4 == 0:
                            hv = 2 if i == 0 else 1
                            qkproj_slice(0, i // 4, 0, w_cur[0], halves=hv)
                            qkproj_slice(0, i // 4, 1, w_cur[1], halves=hv)
                        vproj_t(0, i)
                    if p == NPAIR - 1 and c == 0:
                        while carry and carry[0][1] <= i:
                            carry.pop(0)[0]()
                    score_unit(p, c, i, pt_cur)
                    if i >= 2:
                        av_ktile(p, av_t, 0, i - 2, pt_cur,
                                 first=(i == 2), last=False)
                    if i >= 3:
                        av_ktile(p, av_t, 1, i - 3, pt_cur,
                                 first=(i == 3), last=False)
                    if i >= 1 and fillers:
                        target = (total * i) // (NKT - 1)
                        while fillers and spent < target:
                            cn, f = fillers.pop(0)
                            f()
                            spent += cn
                for k in (NKT - 2, NKT - 1):
                    av_ktile(p, av_t, 0, k, pt_cur, first=False,
                             last=(k == NKT - 1))
                for k in (NKT - 3, NKT - 2, NKT - 1):
                    av_ktile(p, av_t, 1, k, pt_cur, first=False,
                             last=(k == NKT - 1))
                for _, f in fillers:
                    f()
                norm_chunk(p, c, av_t, rcp_t)
                if p == NPAIR - 1 and c < NCH - 1:
                    pend_out = c

            while emitted < n_items:
                items[emitted][1]()
                emitted += 1
            w_cur = w_nxt

        # tail: the last chunk's pair-3 transpose + out-projection
        for u in last_units[NPAIR - 1:]:
            u()

        if _DEBUG:
            nc.sync.dma_start(out=dbg_qkT[:], in_=qkT[:])
            nc.sync.dma_start(out=dbg_v[:], in_=v_sb[:])
            nc.sync.dma_start(out=dbg_o[:], in_=o_sb[:])

    nc.compile()
    return nc


def _get_nc():
    if "nc" not in _CACHE:
        _CACHE["nc"] = _build()
    return _CACHE["nc"]


def _bf16(a):
    import ml_dtypes
    return np.ascontiguousarray(
        np.asarray(a, dtype=np.float32).astype(ml_dtypes.bfloat16))


def make_in_map(xT, wqT, wkT, wvT, woT, bq, bk, bv):
    """Pack one core's inputs into the kernel's tiled DRAM layouts.

    xT: [D, S]; wqT/wkT/wvT: [D, FH] (W sections transposed);
    woT: [FH, D] (out_w columns transposed); biases: [FH].
    """
    D, FH, ND, NPAIR, NH, FHA, NCH, CH = (
        _D, _FH, _ND, _NPAIR, _NH, _FHA, _NCH, _CH)
    # augment v with a per-head ones column: wv gets zero columns, bv gets
    # 1.0 entries -> the broadcast bias add installs the ones column, whose
    # AV accumulation yields the softmax denominators for free
    wva = np.zeros((D, FHA), dtype=np.float32)
    bva = np.zeros((1, FHA), dtype=np.float32)
    for h in range(NH):
        wva[:, h * 65:h * 65 + 64] = np.asarray(wvT)[:, h * 64:(h + 1) * 64]
        bva[0, h * 65:h * 65 + 64] = np.asarray(bv)[h * 64:(h + 1) * 64]
        bva[0, h * 65 + 64] = 1.0
    return {
        "xP": _bf16(np.asarray(xT).reshape(ND, 128, NCH, CH).transpose(2, 1, 0, 3)),
        "wq": _bf16(np.asarray(wqT).reshape(ND, 128, NPAIR, 128).transpose(2, 1, 0, 3)),
        "wk": _bf16(np.asarray(wkT).reshape(ND, 128, NPAIR, 128).transpose(2, 1, 0, 3)),
        "wv": _bf16(wva.reshape(ND, 128, FHA).transpose(1, 0, 2)),
        "wo": _bf16(np.asarray(woT).reshape(NPAIR, 128, D).transpose(1, 0, 2)),
        "bq": np.ascontiguousarray(
            np.asarray(bq, dtype=np.float32).reshape(NPAIR, 128).T),
        "bk": np.ascontiguousarray(
            np.asarray(bk, dtype=np.float32).reshape(NPAIR, 128).T),
        "bv": _bf16(bva),
        "idn": _bf16(np.eye(128)),
    }


def unpack_out(outp_tiled):
    """[NCH, ND, 128, CH] tiled partial -> [D, S] float32."""
    a = np.asarray(outp_tiled, dtype=np.float32)
    return a.transpose(1, 2, 0, 3).reshape(_D, _S)


def _shard_inputs(x, in_proj_weight, in_proj_bias, out_w):
    w = np.asarray(in_proj_weight)
    b = np.asarray(in_proj_bias)
    ow = np.asarray(out_w)
    in_maps = []
    for c in range(_NCORES):
        bi, g = divmod(c, 2)
        sl = slice(g * _FH, (g + 1) * _FH)
        in_maps.append(make_in_map(
            xT=np.asarray(x[bi]).T,
            wqT=w[0 * _D:1 * _D][sl].T,
            wkT=w[1 * _D:2 * _D][sl].T,
            wvT=w[2 * _D:3 * _D][sl].T,
            woT=ow[:, sl].T,
            bq=b[0 * _D:1 * _D][sl],
            bk=b[1 * _D:2 * _D][sl],
            bv=b[2 * _D:3 * _D][sl],
        ))
    return in_maps


LAST_RESULTS = None


def kernel(x, in_proj_weight, in_proj_bias, out_w, out_b):
    global LAST_RESULTS
    from concourse.bass_utils import run_bass_kernel_spmd
    import os

    nc = _get_nc()
    in_maps = _shard_inputs(x, in_proj_weight, in_proj_bias, out_w)
    trace = os.environ.get("BASS_TRACE", "0") not in ("", "0")
    res = run_bass_kernel_spmd(
        nc, in_maps, core_ids=list(range(_NCORES)), trace=trace
    )
    LAST_RESULTS = res
    out_b = np.asarray(out_b, dtype=np.float32)
    out = np.empty((_B, _S, _D), dtype=np.float32)
    for b in range(_B):
        part = (unpack_out(res.results[2 * b]["outp"])
                + unpack_out(res.results[2 * b + 1]["outp"]))
        out[b] = part.T + out_b
    return out


# revision 36
# speedup vs baseline: 1.0141x; 1.0141x over previous
"""Multi-head self-attention (B=4, S=2048, D=1024, H=16) on 8 NeuronCores.

Sharding: data-parallel over batch (4 groups) x tensor-parallel over heads
(2 groups of 8 heads).  Core c handles batch b=c//2, head-group g=c%2.
Each core computes its 8 heads' attention plus a partial out-projection;
the host sums the two partials per batch, transposes, adds out_b.

v2 design (vs v1): everything in bf16 (plenty of margin vs the 2e-2 gate),
no DRAM staging, and a "flipped" AV matmul that halves the PE rows:

  - scores^T per head-pair via row-packed K=64 matmuls, psum tile
    [128 keys, 2*CH] holds both heads; ONE exp per ktile ([128, 1024]
    activation, scale=1/8 folded) -> pt [keys, q] bf16 in SBUF
  - AV flipped: stationary = pt q-block [128 keys, 128 q], moving = v_aug
    [128 keys, 65] (64 v dims + ones column) -> psum [128 q, 65] per
    (qblock, head), accumulated over the 16 ktiles.  65 moving rows per
    ktile instead of 128 q rows: ~2x fewer PE cycles than v1's AV.  The
    ones column (installed by a broadcast bias add on the v projection)
    lands the softmax denominator in psum column 64, per PARTITION
    (= per q), so normalization is a native tensor_scalar multiply.
  - PSUM accumulation groups must run ONE AT A TIME per psum bank
    (a start=True while another group is open in the same bank resets the
    bank - verified on hw).  So AV for chunk (c) runs as a post-pass
    (one (qblock, head) group after another) in the NEXT chunk's window,
    with pt double-buffered.
  - o comes out [q, feat]; out-projection needs o^T, done with cheap PE
    transposes (128 rows each) through PSUM.
  - schedule: pair-outer; pair p+1's q/k/v projections and chunk c-1's
    out-projection interleave into pair p's ACT-bound attention windows.
    ACT does only the 256 exps (~266us); PE ~280us; DVE does all
    PSUM->SBUF moves + bias/normalize (~110us).  gpsimd cannot read PSUM
    (walrus codegen fails) so DVE carries the copies.
"""

import numpy as np

_B, _S, _D, _H = 4, 2048, 1024, 16
_FH = 512        # local feature dims per core (8 heads x 64)
_ND = _D // 128  # contraction tiles
_NPAIR = 4       # head pairs (2 heads of 64 -> 128 features)
_NKT = _S // 128 # key tiles
_CH = 512        # q chunk
_NCH = _S // _CH
_NQB = _CH // 128
_NH = 8          # local heads
_FHA = _NH * 65  # v width incl. per-head ones column
_NCORES = 8

_CACHE = {}
_DEBUG = False  # adds qkT/v/o debug outputs to the kernel


def _build():
    import concourse.bass as bass
    import concourse.bacc as bacc
    import concourse.tile as tile
    import concourse.mybir as mybir
    from contextlib import ExitStack

    f32 = mybir.dt.float32
    bf16 = mybir.dt.bfloat16
    Exp = mybir.ActivationFunctionType.Exp
    D, S, FH, ND, NPAIR, NKT, CH, NCH, NQB, FHA = (
        _D, _S, _FH, _ND, _NPAIR, _NKT, _CH, _NCH, _NQB, _FHA)

    nc = bacc.Bacc("TRN2", target_bir_lowering=False, debug=False)

    xP_d = nc.dram_tensor("xP", [NCH, 128, ND, CH], bf16, kind="ExternalInput")
    wq_d = nc.dram_tensor("wq", [NPAIR, 128, ND, 128], bf16, kind="ExternalInput")
    wk_d = nc.dram_tensor("wk", [NPAIR, 128, ND, 128], bf16, kind="ExternalInput")
    wv_d = nc.dram_tensor("wv", [128, ND, FHA], bf16, kind="ExternalInput")
    wo_d = nc.dram_tensor("wo", [128, NPAIR, D], bf16, kind="ExternalInput")
    bq_d = nc.dram_tensor("bq", [128, NPAIR], f32, kind="ExternalInput")
    bk_d = nc.dram_tensor("bk", [128, NPAIR], f32, kind="ExternalInput")
    bv_d = nc.dram_tensor("bv", [1, FHA], bf16, kind="ExternalInput")
    idn_d = nc.dram_tensor("idn", [128, 128], bf16, kind="ExternalInput")
    outp_d = nc.dram_tensor("outp", [NCH, ND, 128, CH], bf16, kind="ExternalOutput")
    if _DEBUG:
        dbg_qkT = nc.dram_tensor("dbg_qkT", [128, NPAIR, 2, S], bf16, kind="ExternalOutput")
        dbg_v = nc.dram_tensor("dbg_v", [128, NKT, FHA], bf16, kind="ExternalOutput")
        dbg_o = nc.dram_tensor("dbg_o", [128, S // 128, FH], bf16, kind="ExternalOutput")

    with tile.TileContext(nc) as tc, ExitStack() as top:
        consts = top.enter_context(tc.tile_pool(name="consts", bufs=1))
        ps = top.enter_context(tc.tile_pool(name="ps", bufs=2, space="PSUM"))
        big = top.enter_context(tc.tile_pool(name="big", bufs=1))
        ptp = top.enter_context(tc.tile_pool(name="ptp", bufs=2))
        wst = top.enter_context(tc.tile_pool(name="wst", bufs=2))
        otp = top.enter_context(tc.tile_pool(name="otp", bufs=2))
        stp = top.enter_context(tc.tile_pool(name="stp", bufs=3))
        rcpp = top.enter_context(tc.tile_pool(name="rcpp", bufs=2))

        xT_sb = big.tile([128, ND, S], bf16)
        qkT = big.tile([128, NPAIR, 2, S], bf16)  # [feat%128, pair, q/k, t]
        v_sb = big.tile([128, NKT, FHA], bf16)    # [token%128, ktile, head*65]
        o_sb = big.tile([128, S // 128, FH], bf16)  # [q%128, qblock, feat]
        wv_sb = big.tile([128, ND, FHA], bf16)
        wo_sb = big.tile([128, NPAIR, D], bf16)

        def load_w(p):
            wq_sb = wst.tile([128, ND, 128], bf16, tag="wq")
            nc.sync.dma_start(out=wq_sb, in_=wq_d[p])
            wk_sb = wst.tile([128, ND, 128], bf16, tag="wk")
            nc.sync.dma_start(out=wk_sb, in_=wk_d[p])
            return wq_sb, wk_sb

        # DMA order tuned for the warmup critical path: the first qkproj
        # half-slice needs the first half of x slice 0 + wq0/wk0; the
        # first vproj needs wv
        w_cur = load_w(0)
        nc.sync.dma_start(out=xT_sb[:, :, 0:CH // 2], in_=xP_d[0][:, :, 0:CH // 2])
        nc.sync.dma_start(out=xT_sb[:, :, CH // 2:CH], in_=xP_d[0][:, :, CH // 2:CH])
        nc.sync.dma_start(out=wv_sb, in_=wv_d[:])
        bqk_sb = consts.tile([128, 2 * NPAIR], f32)
        nc.sync.dma_start(out=bqk_sb[:, 0:NPAIR], in_=bq_d[:])
        nc.sync.dma_start(out=bqk_sb[:, NPAIR:2 * NPAIR], in_=bk_d[:])
        # v bias broadcast to all partitions (includes the 1.0 ones-column
        # entries that seed the softmax-denominator trick)
        bvb_sb = consts.tile([128, FHA], bf16)
        nc.sync.dma_start(out=bvb_sb, in_=bv_d[:].to_broadcast([128, FHA]))
        for ts in range(1, NCH):
            nc.sync.dma_start(
                out=xT_sb[:, :, ts * CH:(ts + 1) * CH], in_=xP_d[ts])
        idn_sb = consts.tile([128, 128], bf16)
        nc.sync.dma_start(out=idn_sb, in_=idn_d[:])
        nc.sync.dma_start(out=wo_sb, in_=wo_d[:])
        # dummy exp so the ACT table set loads during the ramp
        warm = consts.tile([1, 8], f32)
        nc.vector.memset(warm, 0.0)
        nc.scalar.activation(out=warm, in_=warm, func=Exp)

        def qkproj_slice(p, j, which, w_sb, halves=1):
            pps = ps.tile([128, CH], f32, tag="mix")
            hw_ = CH // halves
            for hf in range(halves):
                for d in range(ND):
                    nc.tensor.matmul(
                        pps[:, hf * hw_:(hf + 1) * hw_],
                        lhsT=w_sb[:, d, :],
                        rhs=xT_sb[:, d, j * CH + hf * hw_:
                                  j * CH + (hf + 1) * hw_],
                        start=(d == 0),
                        stop=(d == ND - 1),
                    )
            nc.vector.tensor_scalar_add(
                out=qkT[:, p, which, j * CH:(j + 1) * CH],
                in0=pps,
                scalar1=bqk_sb[:, which * NPAIR + p:which * NPAIR + p + 1],
            )

        def vproj_t(p, t):
            vps = ps.tile([128, 130], f32, tag="mix")
            for d in range(ND):
                nc.tensor.matmul(
                    vps,
                    lhsT=xT_sb[:, d, t * 128:(t + 1) * 128],
                    rhs=wv_sb[:, d, p * 130:(p + 1) * 130],
                    start=(d == 0),
                    stop=(d == ND - 1),
                )
            nc.vector.tensor_add(
                out=v_sb[:, t, p * 130:(p + 1) * 130],
                in0=vps,
                in1=bvb_sb[:, p * 130:(p + 1) * 130],
            )

        def score_unit(p, c, i, pt_cur):
            sAB = ps.tile([128, 2 * CH], f32, tag="sab")
            nc.tensor.matmul(
                sAB[:, 0:CH],
                lhsT=qkT[0:64, p, 1, i * 128:(i + 1) * 128],
                rhs=qkT[0:64, p, 0, c * CH:(c + 1) * CH],
                start=True, stop=True,
                tile_position=(0, 0),
            )
            nc.tensor.matmul(
                sAB[:, CH:2 * CH],
                lhsT=qkT[64:128, p, 1, i * 128:(i + 1) * 128],
                rhs=qkT[64:128, p, 0, c * CH:(c + 1) * CH],
                start=True, stop=True,
                tile_position=(64, 0),
            )
            nc.scalar.activation(
                out=pt_cur[:, i, :], in_=sAB, func=Exp, scale=0.125)

        def av_ktile(p, av_t, half, i, pt_cur, first, last):
            """AV matmuls for one psum bank (av01 or av23) at ktile i.

            The bank runs ONE accumulation context per chunk: start=True
            only on the bank's first write (resets the bank's
            written-bitmap; untouched regions then store on first touch,
            accumulate after -- verified on hw), stop on its last.
            """
            for qbl in range(2):
                qb = half * 2 + qbl
                for h in range(2):
                    nc.tensor.matmul(
                        av_t[half][:, qbl, h * 65:(h + 1) * 65],
                        lhsT=pt_cur[:, i, h * CH + qb * 128:
                                    h * CH + (qb + 1) * 128],
                        rhs=v_sb[:, i, p * 130 + h * 65:
                                 p * 130 + (h + 1) * 65],
                        start=(first and qbl == 0 and h == 0),
                        stop=(last and qbl == 1 and h == 1),
                        skip_group_check=True,
                    )

        def norm_chunk(p, c, av_t, rcp_t):
            for half in range(2):
                for h in range(2):
                    nc.vector.reciprocal_approx_fast(
                        out=rcp_t[:, half, 2 * h:2 * h + 1],
                        in_=av_t[half][:, 0:1, 64 + 65 * h:65 + 65 * h],
                    )
                    nc.vector.reciprocal_approx_fast(
                        out=rcp_t[:, half, 2 * h + 1:2 * h + 2],
                        in_=av_t[half][:, 1:2, 64 + 65 * h:65 + 65 * h],
                    )
                for qbl in range(2):
                    qb = half * 2 + qbl
                    for h in range(2):
                        nc.vector.tensor_scalar_mul(
                            out=o_sb[:, c * NQB + qb,
                                     p * 128 + h * 64:p * 128 + (h + 1) * 64],
                            in0=av_t[half][:, qbl, h * 65:h * 65 + 64],
                            scalar1=rcp_t[:, half,
                                          2 * h + qbl:2 * h + qbl + 1],
                        )

        def emit_out_units(c):
            """Transposes + out-projection for chunk c, as thunks.

            tps units for fb<3 depend only on pairs 0-2 (whose chunk-c
            norms ran long ago); fb==3 waits on pair 3's norm.
            """
            oT = otp.tile([128, NPAIR, CH], bf16, tag="ot")

            def tps_unit(fb):
                def go():
                    tps = ps.tile([128, NQB, 128], bf16, tag="mix")
                    for qb in range(NQB):
                        nc.tensor.transpose(
                            out=tps[:, qb, :],
                            in_=o_sb[:, c * NQB + qb, fb * 128:(fb + 1) * 128],
                            identity=idn_sb,
                        )
                    nc.vector.tensor_copy(out=oT[:, fb, :], in_=tps)
                return go

            def ops_unit(et, on_act):
                def go():
                    ops = ps.tile([128, CH], f32, tag="mix")
                    for pb in range(NPAIR):
                        nc.tensor.matmul(
                            ops,
                            lhsT=wo_sb[:, pb, et * 128:(et + 1) * 128],
                            rhs=oT[:, pb, :],
                            start=(pb == 0),
                            stop=(pb == NPAIR - 1),
                        )
                    st = stp.tile([128, CH], bf16, tag="st")
                    if on_act:
                        nc.scalar.copy(out=st, in_=ops)
                    else:
                        nc.vector.tensor_copy(out=st, in_=ops)
                    nc.sync.dma_start(out=outp_d[c, et], in_=st)
                return go

            units = [tps_unit(fb) for fb in range(NPAIR)]
            # toward the tail ACT gains slack: alternate the psum->sbuf
            # copies between ACT and DVE so the DVE queue (which also
            # carries the final norm) doesn't serialize the drain
            units += [ops_unit(et, c >= NCH - 2 and et % 2 == 0)
                      for et in range(ND)]
            return units

        # ----- main: pair-outer, chunk-inner.  Per ktile: scores+exp for
        # ktile i, AV for ktile i-1 (bank av01) and i-2 (bank av23) -- the
        # lag keeps the in-order PE from blocking on the just-issued exp.
        # Projection / out-projection filler work is cost-paced between
        # ktiles so the PE never starves while pacing behind ACT. -----
        pend_out = None    # chunk index awaiting emit_out (pair 3)
        carry = []         # (thunk, deadline-ktile) for window (p, 0)
        own_q = []         # next pair's late q-side slices, for window (p, 1)
        w_nxt = None
        for p in range(NPAIR):
            next_carry = []
            next_own = []
            if p + 1 < NPAIR:
                w_nxt = load_w(p + 1)
                vp = p + 1

                def mk_qk(j, w, ws=None, pp=None):
                    pp = vp if pp is None else pp
                    ws = w_nxt[w] if ws is None else ws
                    return lambda: qkproj_slice(pp, j, w, ws)

                # balance the projection load for pair p+1 across windows:
                # - v and the j0 q/k slices spread over pair p's windows
                #   (needed from (p+1, 0) ktile 0 on)
                # - k-sides j1..j3 + the j1 q-side carry into window
                #   (p+1, 0) itself, emitted before the ktile that first
                #   consumes each k-slice
                # - q-sides j2/j3 are only needed from window (p+1, j) on:
                #   they run as leading fillers of window (p+1, 1)
                items = [(500, (lambda pp=vp, t=t: vproj_t(pp, t)))
                         for t in range(NKT)]
                items += [(1740, mk_qk(0, 0)), (1740, mk_qk(0, 1))]
                if vp == NPAIR - 1:
                    # pair 3's windows are full with out-proj work: its
                    # late q/k slices spread over pair 2 instead
                    items += [(1740, mk_qk(2, 0)), (1740, mk_qk(3, 0)),
                              (1740, mk_qk(2, 1)), (1740, mk_qk(3, 1))]
                    next_carry = [(mk_qk(1, 1), 2), (mk_qk(1, 0), 13)]
                else:
                    next_carry = [(mk_qk(1, 1), 2), (mk_qk(2, 1), 6),
                                  (mk_qk(3, 1), 10), (mk_qk(1, 0), 13)]
                    next_own = [mk_qk(2, 0), mk_qk(3, 0)]
            else:
                items = []
            n_items = len(items)
            emitted = 0
            nwin = NCH if p > 0 else NCH - 1
            denom = max(1, nwin * NKT - 6)
            it_count = 0

            for c in range(NCH):
                # fillers: (cost_ns, thunk) of PE work to spread between
                # the score units
                fillers = []
                if c == 1 and own_q:
                    fillers += [(1740, u) for u in own_q]
                    own_q = []
                if pend_out is not None:
                    fillers += [(220, u) if k < NPAIR else (870, u)
                                for k, u in enumerate(emit_out_units(pend_out))]
                    pend_out = None
                last_units = None
                if p == NPAIR - 1 and c == NCH - 1:
                    # the final chunk's fb 0..2 transposes only need pairs
                    # 0-2 (normalized long ago): run them in this window,
                    # leaving just tps3 + out-proj for the tail
                    last_units = emit_out_units(NCH - 1)
                    fillers += [(220, u) for u in last_units[:NPAIR - 1]]
                if items and not (p == 0 and c == 0):
                    it_count += NKT
                    want = min(n_items, (it_count * n_items) // denom)
                    while emitted < want:
                        fillers.append(items[emitted])
                        emitted += 1

                pt_cur = ptp.tile([128, NKT, 2 * CH], bf16, tag="pt")
                av_t = [ps.tile([128, 2, 130], f32, tag="av", name=f"av{h}")
                        for h in range(2)]
                rcp_t = rcpp.tile([128, 2, 4], f32, tag="rcp")
                total = sum(cn for cn, _ in fillers)
                spent = 0
                for i in range(NKT):
                    if p == 0 and c == 0:
                        # inline projections for pair 0, aligned with the
                        # ktile order scores consume them in; the first
                        # slices run half-width so the leading matmuls only
                        # wait on the first half-slice x DMA; q-sides j>=1
                        # are deferred (only needed from window (0,j) on)
                        if i == 0:
                            qkproj_slice(0, 0, 0, w_cur[0], halves=2)
                            qkproj_slice(0, 0, 1, w_cur[1], halves=2)
                            own_q = [
                                (lambda jj=jj, ws=w_cur[0]:
                                 qkproj_slice(0, jj, 0, ws))
                                for jj in (2, 3)]
                        elif i % 4 == 0:
                            qkproj_slice(0, i // 4, 1, w_cur[1])
                        elif i == 13:
                            qkproj_slice(0, 1, 0, w_cur[0])
                        vproj_t(0, i)
                    if c == 0:
                        while carry and carry[0][1] <= i:
                            carry.pop(0)[0]()
                    score_unit(p, c, i, pt_cur)
                    if i >= 2:
                        av_ktile(p, av_t, 0, i - 2, pt_cur,
                                 first=(i == 2), last=False)
                    if i >= 3:
                        av_ktile(p, av_t, 1, i - 3, pt_cur,
                                 first=(i == 3), last=False)
                    if i >= 1 and fillers:
                        target = (total * i) // (NKT - 1)
                        while fillers and spent < target:
                            cn, f = fillers.pop(0)
                            f()
                            spent += cn
                for k in (NKT - 2, NKT - 1):
                    av_ktile(p, av_t, 0, k, pt_cur, first=False,
                             last=(k == NKT - 1))
                for k in (NKT - 3, NKT - 2, NKT - 1):
                    av_ktile(p, av_t, 1, k, pt_cur, first=False,
                             last=(k == NKT - 1))
                for _, f in fillers:
                    f()
                norm_chunk(p, c, av_t, rcp_t)
                if p == NPAIR - 1 and c < NCH - 1:
                    pend_out = c

            while emitted < n_items:
                items[emitted][1]()
                emitted += 1
            w_cur = w_nxt
            carry = next_carry
            own_q = next_own

        # tail: the last chunk's pair-3 transpose + out-projection
        for u in last_units[NPAIR - 1:]:
            u()

        if _DEBUG:
            nc.sync.dma_start(out=dbg_qkT[:], in_=qkT[:])
            nc.sync.dma_start(out=dbg_v[:], in_=v_sb[:])
            nc.sync.dma_start(out=dbg_o[:], in_=o_sb[:])

    nc.compile()
    return nc


def _get_nc():
    if "nc" not in _CACHE:
        _CACHE["nc"] = _build()
    return _CACHE["nc"]


def _bf16(a):
    import ml_dtypes
    return np.ascontiguousarray(
        np.asarray(a, dtype=np.float32).astype(ml_dtypes.bfloat16))


def make_in_map(xT, wqT, wkT, wvT, woT, bq, bk, bv):
    """Pack one core's inputs into the kernel's tiled DRAM layouts.

    xT: [D, S]; wqT/wkT/wvT: [D, FH] (W sections transposed);
    woT: [FH, D] (out_w columns transposed); biases: [FH].
    """
    D, FH, ND, NPAIR, NH, FHA, NCH, CH = (
        _D, _FH, _ND, _NPAIR, _NH, _FHA, _NCH, _CH)
    # augment v with a per-head ones column: wv gets zero columns, bv gets
    # 1.0 entries -> the broadcast bias add installs the ones column, whose
    # AV accumulation yields the softmax denominators for free
    wva = np.zeros((D, FHA), dtype=np.float32)
    bva = np.zeros((1, FHA), dtype=np.float32)
    for h in range(NH):
        wva[:, h * 65:h * 65 + 64] = np.asarray(wvT)[:, h * 64:(h + 1) * 64]
        bva[0, h * 65:h * 65 + 64] = np.asarray(bv)[h * 64:(h + 1) * 64]
        bva[0, h * 65 + 64] = 1.0
    return {
        "xP": _bf16(np.asarray(xT).reshape(ND, 128, NCH, CH).transpose(2, 1, 0, 3)),
        "wq": _bf16(np.asarray(wqT).reshape(ND, 128, NPAIR, 128).transpose(2, 1, 0, 3)),
        "wk": _bf16(np.asarray(wkT).reshape(ND, 128, NPAIR, 128).transpose(2, 1, 0, 3)),
        "wv": _bf16(wva.reshape(ND, 128, FHA).transpose(1, 0, 2)),
        "wo": _bf16(np.asarray(woT).reshape(NPAIR, 128, D).transpose(1, 0, 2)),
        "bq": np.ascontiguousarray(
            np.asarray(bq, dtype=np.float32).reshape(NPAIR, 128).T),
        "bk": np.ascontiguousarray(
            np.asarray(bk, dtype=np.float32).reshape(NPAIR, 128).T),
        "bv": _bf16(bva),
        "idn": _bf16(np.eye(128)),
    }


def unpack_out(outp_tiled):
    """[NCH, ND, 128, CH] tiled partial -> [D, S] float32."""
    a = np.asarray(outp_tiled, dtype=np.float32)
    return a.transpose(1, 2, 0, 3).reshape(_D, _S)


def _shard_inputs(x, in_proj_weight, in_proj_bias, out_w):
    w = np.asarray(in_proj_weight)
    b = np.asarray(in_proj_bias)
    ow = np.asarray(out_w)
    in_maps = []
    for c in range(_NCORES):
        bi, g = divmod(c, 2)
        sl = slice(g * _FH, (g + 1) * _FH)
        in_maps.append(make_in_map(
            xT=np.asarray(x[bi]).T,
            wqT=w[0 * _D:1 * _D][sl].T,
            wkT=w[1 * _D:2 * _D][sl].T,
            wvT=w[2 * _D:3 * _D][sl].T,
            woT=ow[:, sl].T,
            bq=b[0 * _D:1 * _D][sl],
            bk=b[1 * _D:2 * _D][sl],
            bv=b[2 * _D:3 * _D][sl],
        ))
    return in_maps


LAST_RESULTS = None


def kernel(x, in_proj_weight, in_proj_bias, out_w, out_b):
    global LAST_RESULTS
    from concourse.bass_utils import run_bass_kernel_spmd
    import os

    nc = _get_nc()
    in_maps = _shard_inputs(x, in_proj_weight, in_proj_bias, out_w)
    trace = os.environ.get("BASS_TRACE", "0") not in ("", "0")
    res = run_bass_kernel_spmd(
        nc, in_maps, core_ids=list(range(_NCORES)), trace=trace
    )
    LAST_RESULTS = res
    out_b = np.asarray(out_b, dtype=np.float32)
    out = np.empty((_B, _S, _D), dtype=np.float32)
    for b in range(_B):
        part = (unpack_out(res.results[2 * b]["outp"])
                + unpack_out(res.results[2 * b + 1]["outp"]))
        out[b] = part.T + out_b
    return out
